# revision 38
# baseline (speedup 1.0000x reference)
"""Trainium2 Bass kernel for nn_Loss_40510131536268.

Algorithm
---------
The reference is a T-step normalized forward recursion over a fixed sparse
graph (E=16384 edges on V=2048 nodes), batched over B=32:

    log_C   = logsumexp(log_prev over out-nodes)
    prop    = exp(log_prev[:, out_idxs] - log_C)
    combined= scatter_add(prop -> in_idxs)
    log_curr= log_safe(combined) + x_t
    result  = log(sum over end nodes of exp(log_curr)) + sum(log_C)  at t+1==len

In probability space the per-step normalization by C cancels exactly in the
final result, so the recursion linearizes to

    U_t = (U_{t-1} @ A) * X_t        A[u,w] = #edges u->w,  X_t = exp(x_t)

with result[b] = log( sum_v U_{L-1}[b,v] * end_w[v] ) plus exact bookkeeping
for the per-step scales folded into X to keep bf16 in range.  The EPS clamps
of the reference only affect mass at relative level e^-64 — invisible here.

Device schedule (per core, data-parallel over B: 4 batch rows per core)
-----------------------------------------------------------------------
State lives v-major in SBUF as ur[32g+r, 32j+b] = U[b, 512g+32j+r] so the
matmul lhsT tile for contraction chunk k is simply ur[:, 32k:32k+4] and the
per-step state rebuild is a single DVE StreamTranspose (32x32-block-local)
instead of 16 PE transposes + copies.  A's rows are host-permuted to match
(row 32g+r of chunk k is source node 512g+32k+r).

Per step:
  - 64 accumulating matmuls [K=128, M=4, N=512] k-outer/group-inner with
    tile_position=(0,32g): 4 concurrent column-group streams — the only PE
    work (the matmul-only skeleton measures ~3.0us/step).
  - 4 DVE StreamTranspose pieces [128,128] straight out of PSUM (junk rows
    land in never-read junk columns), then 4 DVE multiplies apply X_t on
    only the 64 valid columns per piece (~1.7us DVE work, mostly hidden).
  - 1 scalar-engine copy packs the valid columns into a history staging
    tile; one DMA per 8-step block stores it, one DMA loads 8 steps of X.
Measured ~4.0-4.2us/step (vs 4.9 for the v1 dense+PE-transpose baseline and
13.3 sustained); N=256 splits and finer transpose pieces both measured worse.
Host: exp/scaling prep, final gather of E_t = U_t . end_w at t = L_b - 1.
No collectives; 8 cores each run an independent batch shard.
"""

import numpy as np
import ml_dtypes

bf16 = ml_dtypes.bfloat16

V, B, T, E, S = 2048, 32, 256, 16384, 128
NCORES = 8
BL = B // NCORES        # 4 batch rows per core
NK = V // 128           # 16 contraction tiles
NG = 4                  # column-tile groups / output chunks of 512
XBATCH = 8              # steps per DMA batch
EPS = float(np.exp(-64.0))

_PROGRAM_CACHE = {}


def _split_multi_waits(nc):
    """walrus in this toolchain rejects compute instructions carrying more
    than one semaphore wait ("Too many sync wait commands").  Split extra
    waits onto no-op instructions inserted immediately before, on the same
    engine (engine-local program order preserves the gating semantics)."""
    import concourse.mybir as mybir

    skip = (
        mybir.InstCall,
        mybir.InstUnconditionalBranch,
        mybir.InstCompareAndBranch,
        mybir.InstIndirectBranch,
        mybir.InstHalt,
    )
    for f in nc.m.functions:
        for blk in f.blocks:
            out = []
            changed = False
            for inst in blk.instructions:
                si = inst.sync_info
                if (
                    si is not None
                    and si.on_wait
                    and len(si.on_wait) > 1
                    and not isinstance(inst, skip)
                ):
                    waits = list(si.on_wait)
                    for w in waits[:-1]:
                        out.append(
                            mybir.InstNoOp(
                                name=nc.get_next_instruction_name(),
                                engine=inst.engine,
                                ins=[],
                                outs=[],
                                bass_nofuse=True,
                                sync_info=mybir.SyncInfo(on_wait=[w], on_update=[]),
                            )
                        )
                    inst.sync_info = mybir.SyncInfo(
                        on_wait=[waits[-1]], on_update=list(si.on_update or [])
                    )
                    changed = True
                out.append(inst)
            if changed:
                blk.instructions = out


def build_program(n_steps, split_waits=True, outer_reps=1, timing_mode=False,
                  surrogate=False, nsplit=1, nodve=False, probe=False):
    """Build the SPMD Bass/Tile program (identical on all 8 cores).

    outer_reps: wrap the step loop in a hardware For_i (timing use only).
    timing_mode: shrink u_hist to one block (all blocks store to the same
        region) so the donated-zero upload per invocation is tiny.
    surrogate: replace StreamTranspose with a same-shape DVE tensor_copy —
        timing-equivalent, and unlike StreamTranspose it survives walrus
        codegen inside a hardware For_i loop ("ISA wrong length" bug)."""
    import concourse.bass as bass
    import concourse.mybir as mybir
    from concourse.tile import TileContext

    f32 = mybir.dt.float32
    b16 = mybir.dt.bfloat16

    nc = bass.Bass()
    a_in = nc.declare_dram_parameter("a_rhs", [128, NK * V], b16, isOutput=False)
    u0_in = nc.declare_dram_parameter("u0t", [128, 512], b16, isOutput=False)
    xs_in = nc.declare_dram_parameter("xs", [n_steps * 128, 64], b16, isOutput=False)
    uh_rows = (XBATCH if timing_mode else n_steps) * 128
    uh_out = nc.declare_dram_parameter("u_hist", [uh_rows, 64], b16, isOutput=True)

    nb = (n_steps + XBATCH - 1) // XBATCH

    with TileContext(nc) as tc:
        with (
            tc.tile_pool(name="const", bufs=1) as cpool,
            tc.tile_pool(name="xv", bufs=1) as xpool,
            tc.tile_pool(name="ur", bufs=3) as urpool,
            tc.tile_pool(name="un", bufs=3) as unpool,
            tc.tile_pool(name="st", bufs=2) as stpool,
            tc.tile_pool(name="pr", bufs=3, space="PSUM") as prpool,
        ):
            a_sb = cpool.tile([128, NK * V], b16, tag="a_sb")
            nc.gpsimd.dma_start(a_sb[:, :], a_in[:, :])
            u_init = cpool.tile([128, 512], b16, tag="u_init")
            nc.gpsimd.dma_start(u_init[:, :], u0_in[:, :])

            # X staging: compact v-major layout, [128, 64] per step
            xr_bufs = [
                xpool.tile([128, XBATCH * 64], b16, tag=f"xr{p}", name=f"xr{p}")
                for p in range(2)
            ]

            state = {"prev": None}

            def lhsT(i, k):
                if i == 0:
                    return u_init[:, 32 * k : 32 * k + BL]
                return state["prev"][:, 32 * k : 32 * k + BL]

            def load_x(blk):
                i0 = blk * XBATCH
                nsteps_blk = min(XBATCH, n_steps - i0)
                xr = xr_bufs[blk % 2]
                # dst[p, s*64 + c] = xs[(i0+s)*128 + p, c]
                nc.sync.dma_start(
                    xr[:, 0 : nsteps_blk * 64],
                    xs_in[i0 * 128 : (i0 + nsteps_blk) * 128, :].rearrange(
                        "(s p) c -> p s c", s=nsteps_blk
                    ),
                )

            def valid(ap, q):
                # [128, (j in 4q..4q+3, b<4)] view of a [128, 512]-layout AP
                return ap[:, q * 128 : (q + 1) * 128].rearrange(
                    "p (j c) -> p j c", j=4
                )[:, :, 0:BL]

            def body(blk):
                i0 = blk * XBATCH
                nsteps_blk = min(XBATCH, n_steps - i0)
                if blk + 1 < nb:
                    load_x(blk + 1)
                xr = xr_bufs[blk % 2]
                stage = stpool.tile([128, XBATCH * 64], b16, tag="stage")
                def pieces(P, tb, ur, s, ranges, off):
                    # transpose straight out of PSUM (32x32 block-local);
                    # junk rows land in junk columns, never read.  Then apply
                    # X on only the 4 valid columns per k-tile.
                    for a, b in ranges:
                        nc.vector.transpose(
                            tb[:, 32 * a : 32 * b],
                            P[:, 32 * (a - off) : 32 * (b - off)],
                        )
                        nc.vector.tensor_mul(
                            ur[:, 32 * a : 32 * b].rearrange(
                                "p (j c) -> p j c", j=b - a
                            )[:, :, 0:BL],
                            tb[:, 32 * a : 32 * b].rearrange(
                                "p (j c) -> p j c", j=b - a
                            )[:, :, 0:BL],
                            xr[:, s * 64 + BL * a : s * 64 + BL * b].rearrange(
                                "p (j c) -> p j c", j=b - a
                            ),
                        )

                for s in range(nsteps_blk):
                    i = i0 + s
                    tb = unpool.tile([128, 512], f32, tag="tb")
                    ur = urpool.tile([128, 512], b16, tag="ur")
                    # single phase, N=512 streams (a two-phase N=256 column
                    # split that hides the transpose chain measured WORSE:
                    # +415ns round-boundary cost vs ~260ns of hiding gained)
                    P = prpool.tile([128, 512], f32, tag="P")
                    for k in range(NK - 1):
                        for g in range(NG):
                            nc.tensor.matmul(
                                P[32 * g : 32 * g + BL, :],
                                lhsT(i, k),
                                a_sb[:, k * V + g * 512 : k * V + (g + 1) * 512],
                                start=(k == 0),
                                stop=False,
                                tile_position=(0, 32 * g),
                            )
                    # final round split into column halves: the first half's
                    # completion releases the transposes for k-tiles 0..7 one
                    # half-round early, overlapping the chain head with the
                    # PE's last 256 columns of streaming
                    kl = NK - 1
                    for h in range(2):
                        for g in range(NG):
                            nc.tensor.matmul(
                                P[32 * g : 32 * g + BL, h * 256 : (h + 1) * 256],
                                lhsT(i, kl),
                                a_sb[
                                    :,
                                    kl * V + g * 512 + h * 256 :
                                    kl * V + g * 512 + (h + 1) * 256,
                                ],
                                start=False,
                                stop=True,
                                tile_position=(0, 32 * g),
                                skip_group_check=True,
                            )
                    if nodve:
                        if probe:
                            # dangling transposes: read PSUM, no consumers —
                            # isolates DVE-PSUM contention from dependencies
                            for a, b in ((0, 4), (4, 8), (8, 12), (12, 16)):
                                nc.vector.transpose(
                                    tb[:, 32 * a : 32 * b], P[:, 32 * a : 32 * b]
                                )
                        state["prev"] = u_init
                        nc.scalar.copy(
                            stage[:, s * 64 : (s + 1) * 64], u_init[:, 0:64]
                        )
                        continue
                    # uniform 4-piece split measured best: finer head pieces
                    # cost more in DVE op overhead than they save in latency
                    pieces(P, tb, ur, s,
                           ((0, 4), (4, 8), (8, 12), (12, 16)), 0)
                    state["prev"] = ur
                    nc.scalar.copy(
                        stage[:, s * 64 : (s + 1) * 64].rearrange(
                            "p (j c) -> p j c", j=NK
                        ),
                        ur[:, :].rearrange("p (j c) -> p j c", j=NK)[:, :, 0:BL],
                    )
                o0 = 0 if timing_mode else i0 * 128
                # scalar (ACT) DGE queue: gpsimd-queue DMA triggers are Pool
                # ISA instructions and break walrus codegen inside For_i
                nc.scalar.dma_start(
                    uh_out[o0 : o0 + nsteps_blk * 128, :].rearrange(
                        "(s p) c -> p s c", s=nsteps_blk
                    ),
                    stage[:, 0 : nsteps_blk * 64].rearrange(
                        "p (s c) -> p s c", s=nsteps_blk
                    ),
                )

            def run_all():
                load_x(0)
                for blk in range(nb):
                    body(blk)

            if outer_reps == 1:
                run_all()
            else:
                with tc.For_i(0, outer_reps):
                    run_all()
    if split_waits:
        _split_multi_waits(nc)
    return nc


def _prep_host(inputs, n_steps):
    """Host-side preprocessing shared by all cores."""
    x = np.asarray(inputs["extracted_log_probs"], np.float32)   # [V,B,T]
    in_idxs = np.asarray(inputs["in_idxs"]).astype(np.int64)
    out_idxs = np.asarray(inputs["out_idxs"]).astype(np.int64)
    start_idxs = np.asarray(inputs["start_idxs"]).astype(np.int64)
    end_idxs = np.asarray(inputs["end_idxs"]).astype(np.int64)

    xt = np.ascontiguousarray(np.transpose(x, (2, 1, 0)))       # [T,B,V]

    A_cnt = np.zeros((V, V), np.float32)
    np.add.at(A_cnt, (out_idxs, in_idxs), 1.0)

    end_w = np.zeros((V,), np.float32)
    np.add.at(end_w, end_idxs, 1.0)

    start_mask = np.zeros((V,), bool)
    start_mask[start_idxs] = True

    # A tiles, row-permuted to the v-major state layout:
    # a_sb[32g + r, k*V + w] = A[512g + 32k + r, w]
    a_sb = np.ascontiguousarray(
        A_cnt.reshape(NG, NK, 32, V).transpose(0, 2, 1, 3).reshape(128, NK * V)
    ).astype(bf16)

    # U_0 = exp(log_curr0)
    X0 = np.exp(xt[0])                                           # [B,V]
    U0 = np.where(start_mask[None, :], X0, np.float32(EPS)).astype(np.float32)
    U0_16 = U0.astype(bf16)

    # scales sigma_t[b] folded into X' (t = 1..n_steps)
    Xall = np.exp(xt[1 : n_steps + 1])                           # [n,B,V]
    m = Xall.mean(axis=2)                                        # [n,B]
    sigma = (1.0 / (8.0 * m)).astype(np.float32)
    cumlog = np.cumsum(np.log(sigma.astype(np.float64)), axis=0) # [n,B]
    Xs16 = (Xall * sigma[:, :, None]).astype(bf16)               # [n,B,V]

    return dict(a_sb=a_sb, U0_16=U0_16, Xs16=Xs16, cumlog=cumlog, end_w=end_w)


def _core_inputs(prep, core, n_steps):
    bsl = slice(core * BL, (core + 1) * BL)
    # u0t[32g + r, 32j + b] = U0[b, 512g + 32j + r], junk columns zero
    u0c = prep["U0_16"][bsl]                                     # [BL, V]
    u0t = np.zeros((NG, 32, NK, 32), bf16)                       # [g, r, j, c]
    u0t[:, :, :, :BL] = u0c.reshape(BL, NG, NK, 32).transpose(1, 3, 2, 0)
    u0t = np.ascontiguousarray(u0t.reshape(128, 512))
    # xs[(t, 32g + r), 4j + b] = Xs16[t, b, 512g + 32j + r]
    xc = prep["Xs16"][:, bsl, :]                                 # [n, BL, V]
    xs = np.ascontiguousarray(
        xc.reshape(n_steps, BL, NG, NK, 32)
        .transpose(0, 2, 4, 3, 1)                                # [t, g, r, j, b]
        .reshape(n_steps * 128, NK * BL)
    )
    return {"a_rhs": prep["a_sb"], "u0t": u0t, "xs": xs}


def _postprocess(prep, results, target_lengths, n_steps):
    """results: list of per-core out_maps with 'u_hist'."""
    end_w_gjr = prep["end_w"].reshape(NG, NK, 32)                # [g, j, r]
    E_dev = np.zeros((n_steps + 1, B), np.float64)
    # t = 0 from host U0 (bf16-rounded, same as device state precision)
    E_dev[0] = prep["U0_16"].astype(np.float32) @ prep["end_w"]
    for c in range(NCORES):
        uh = np.asarray(results[c]["u_hist"]).reshape(n_steps, NG, 32, NK, BL)
        # E[t, b] = sum_{g,r,j} uh[t, g, r, j, b] * end_w[g, j, r]
        Ec = np.einsum("tgrjb,gjr->tb", uh.astype(np.float32), end_w_gjr)
        E_dev[1:, c * BL : (c + 1) * BL] = Ec
    lengths = np.asarray(target_lengths).astype(np.int64)
    res = np.zeros((B,), np.float64)
    for b in range(B):
        L = int(lengths[b])
        corr = prep["cumlog"][L - 2, b] if L >= 2 else 0.0
        res[b] = np.log(E_dev[L - 1, b]) - corr
    return (-res).astype(np.float32)


def run_on_device(nc, core_maps, **kwargs):
    from concourse.bass_utils import run_bass_kernel_spmd

    return run_bass_kernel_spmd(nc, core_maps, core_ids=list(range(NCORES)), **kwargs)


def kernel(**inputs) -> np.ndarray:
    lengths = np.asarray(inputs["target_lengths"]).astype(np.int64)
    n_steps = max(1, int(lengths.max()) - 1)
    prep = _prep_host(inputs, n_steps)
    core_maps = [_core_inputs(prep, c, n_steps) for c in range(NCORES)]
    last_err = None
    for attempt in range(3):
        try:
            if n_steps not in _PROGRAM_CACHE:
                _PROGRAM_CACHE[n_steps] = build_program(n_steps)
            nc = _PROGRAM_CACHE[n_steps]
            out = run_on_device(nc, core_maps)
            break
        except Exception as e:                      # flaky axon compile path
            last_err = e
            _PROGRAM_CACHE.pop(n_steps, None)
    else:
        raise last_err
    return _postprocess(prep, out.results, inputs["target_lengths"], n_steps)


# revision 39
# speedup vs baseline: 1.1399x; 1.1399x over previous
"""Trainium2 Bass kernel for nn_Loss_40510131536268.

Algorithm
---------
The reference is a T-step normalized forward recursion over a fixed sparse
graph (E=16384 edges on V=2048 nodes), batched over B=32:

    log_C   = logsumexp(log_prev over out-nodes)
    prop    = exp(log_prev[:, out_idxs] - log_C)
    combined= scatter_add(prop -> in_idxs)
    log_curr= log_safe(combined) + x_t
    result  = log(sum over end nodes of exp(log_curr)) + sum(log_C)  at t+1==len

In probability space the per-step normalization by C cancels exactly in the
final result, so the recursion linearizes to

    U_t = (U_{t-1} @ A) * X_t        A[u,w] = #edges u->w,  X_t = exp(x_t)

with result[b] = log( sum_v U_{L-1}[b,v] * end_w[v] ) plus exact bookkeeping
for the per-step scales folded into X to keep bf16 in range.  The EPS clamps
of the reference only affect mass at relative level e^-64 — invisible here.

Device schedule (per core, data-parallel over B: 4 batch rows per core)
-----------------------------------------------------------------------
State lives v-major in SBUF as ur[32g+r, 32j+b] = U[b, 512g+32j+r] so the
matmul lhsT tile for contraction chunk k is simply ur[:, 32k:32k+4] and the
per-step state rebuild is a single DVE StreamTranspose (32x32-block-local)
instead of 16 PE transposes + copies.  A's rows are host-permuted to match
(row 32g+r of chunk k is source node 512g+32k+r).

Per step:
  - 60 accumulating matmuls [K=128, M=4, N=512] k-outer/group-inner with
    tile_position=(0,32g): 4 concurrent column-group streams — the only PE
    work (the matmul-only skeleton measures ~3.0us/step).  The FINAL k-round
    is split into two N=256 column halves so the first half's completion
    releases the transposes for k-tiles 0..7 a half-round early, overlapping
    the dependency-chain head with the PE's last 256 columns of streaming.
  - 4 DVE StreamTranspose pieces [128,128] straight out of PSUM (junk rows
    land in never-read junk columns), then 4 DVE multiplies apply X_t on
    only the 64 valid columns per piece (~1.7us DVE work, mostly hidden).
  - 1 scalar-engine copy packs the valid columns into a history staging
    tile; one DMA per 8-step block stores it, one DMA loads 8 steps of X.
Measured ~4.0us/step overall, ~3.5-4.2 in the fresh-device regime (vs 4.9
for the v1 dense+PE-transpose baseline, 13.3 sustained).  Measured worse and
reverted: all-round N=256 splits, finer/coarser transpose pieces, two-phase
column splits, ACT-engine PSUM drain, splitting the last TWO rounds.  The
residual over the skeleton is inter-engine dependency latency (probe:
dangling transposes cost only +112ns -> no PSUM contention); the remaining
fix is a fused transpose-and-scale custom DVE ucode op.
Host: exp/scaling prep, final gather of E_t = U_t . end_w at t = L_b - 1.
No collectives; 8 cores each run an independent batch shard.
"""

import numpy as np
import ml_dtypes

bf16 = ml_dtypes.bfloat16

V, B, T, E, S = 2048, 32, 256, 16384, 128
NCORES = 8
BL = B // NCORES        # 4 batch rows per core
NK = V // 128           # 16 contraction tiles
NG = 4                  # column-tile groups / output chunks of 512
XBATCH = 8              # steps per DMA batch
EPS = float(np.exp(-64.0))

_PROGRAM_CACHE = {}


def _split_multi_waits(nc):
    """walrus in this toolchain rejects compute instructions carrying more
    than one semaphore wait ("Too many sync wait commands").  Split extra
    waits onto no-op instructions inserted immediately before, on the same
    engine (engine-local program order preserves the gating semantics)."""
    import concourse.mybir as mybir

    skip = (
        mybir.InstCall,
        mybir.InstUnconditionalBranch,
        mybir.InstCompareAndBranch,
        mybir.InstIndirectBranch,
        mybir.InstHalt,
    )
    for f in nc.m.functions:
        for blk in f.blocks:
            out = []
            changed = False
            for inst in blk.instructions:
                si = inst.sync_info
                if (
                    si is not None
                    and si.on_wait
                    and len(si.on_wait) > 1
                    and not isinstance(inst, skip)
                ):
                    waits = list(si.on_wait)
                    for w in waits[:-1]:
                        out.append(
                            mybir.InstNoOp(
                                name=nc.get_next_instruction_name(),
                                engine=inst.engine,
                                ins=[],
                                outs=[],
                                bass_nofuse=True,
                                sync_info=mybir.SyncInfo(on_wait=[w], on_update=[]),
                            )
                        )
                    inst.sync_info = mybir.SyncInfo(
                        on_wait=[waits[-1]], on_update=list(si.on_update or [])
                    )
                    changed = True
                out.append(inst)
            if changed:
                blk.instructions = out


def build_program(n_steps, split_waits=True, outer_reps=1, timing_mode=False,
                  surrogate=False, nsplit=1, nodve=False, probe=False):
    """Build the SPMD Bass/Tile program (identical on all 8 cores).

    outer_reps: wrap the step loop in a hardware For_i (timing use only).
    timing_mode: shrink u_hist to one block (all blocks store to the same
        region) so the donated-zero upload per invocation is tiny.
    surrogate: replace StreamTranspose with a same-shape DVE tensor_copy —
        timing-equivalent, and unlike StreamTranspose it survives walrus
        codegen inside a hardware For_i loop ("ISA wrong length" bug)."""
    import concourse.bass as bass
    import concourse.mybir as mybir
    from concourse.tile import TileContext

    f32 = mybir.dt.float32
    b16 = mybir.dt.bfloat16

    nc = bass.Bass()
    a_in = nc.declare_dram_parameter("a_rhs", [128, NK * V], b16, isOutput=False)
    u0_in = nc.declare_dram_parameter("u0t", [128, 512], b16, isOutput=False)
    xs_in = nc.declare_dram_parameter("xs", [n_steps * 128, 64], b16, isOutput=False)
    uh_rows = (XBATCH if timing_mode else n_steps) * 128
    uh_out = nc.declare_dram_parameter("u_hist", [uh_rows, 64], b16, isOutput=True)

    nb = (n_steps + XBATCH - 1) // XBATCH

    with TileContext(nc) as tc:
        with (
            tc.tile_pool(name="const", bufs=1) as cpool,
            tc.tile_pool(name="xv", bufs=1) as xpool,
            tc.tile_pool(name="ur", bufs=3) as urpool,
            tc.tile_pool(name="un", bufs=3) as unpool,
            tc.tile_pool(name="st", bufs=2) as stpool,
            tc.tile_pool(name="pr", bufs=3, space="PSUM") as prpool,
        ):
            a_sb = cpool.tile([128, NK * V], b16, tag="a_sb")
            nc.gpsimd.dma_start(a_sb[:, :], a_in[:, :])
            u_init = cpool.tile([128, 512], b16, tag="u_init")
            nc.gpsimd.dma_start(u_init[:, :], u0_in[:, :])

            # X staging: compact v-major layout, [128, 64] per step
            xr_bufs = [
                xpool.tile([128, XBATCH * 64], b16, tag=f"xr{p}", name=f"xr{p}")
                for p in range(2)
            ]

            state = {"prev": None}

            def lhsT(i, k):
                if i == 0:
                    return u_init[:, 32 * k : 32 * k + BL]
                return state["prev"][:, 32 * k : 32 * k + BL]

            def load_x(blk):
                i0 = blk * XBATCH
                nsteps_blk = min(XBATCH, n_steps - i0)
                xr = xr_bufs[blk % 2]
                # dst[p, s*64 + c] = xs[(i0+s)*128 + p, c]
                nc.sync.dma_start(
                    xr[:, 0 : nsteps_blk * 64],
                    xs_in[i0 * 128 : (i0 + nsteps_blk) * 128, :].rearrange(
                        "(s p) c -> p s c", s=nsteps_blk
                    ),
                )

            def valid(ap, q):
                # [128, (j in 4q..4q+3, b<4)] view of a [128, 512]-layout AP
                return ap[:, q * 128 : (q + 1) * 128].rearrange(
                    "p (j c) -> p j c", j=4
                )[:, :, 0:BL]

            def body(blk):
                i0 = blk * XBATCH
                nsteps_blk = min(XBATCH, n_steps - i0)
                if blk + 1 < nb:
                    load_x(blk + 1)
                xr = xr_bufs[blk % 2]
                stage = stpool.tile([128, XBATCH * 64], b16, tag="stage")
                def pieces(P, tb, ur, s, ranges, off):
                    # transpose straight out of PSUM (32x32 block-local);
                    # junk rows land in junk columns, never read.  Then apply
                    # X on only the 4 valid columns per k-tile.
                    for a, b in ranges:
                        nc.vector.transpose(
                            tb[:, 32 * a : 32 * b],
                            P[:, 32 * (a - off) : 32 * (b - off)],
                        )
                        nc.vector.tensor_mul(
                            ur[:, 32 * a : 32 * b].rearrange(
                                "p (j c) -> p j c", j=b - a
                            )[:, :, 0:BL],
                            tb[:, 32 * a : 32 * b].rearrange(
                                "p (j c) -> p j c", j=b - a
                            )[:, :, 0:BL],
                            xr[:, s * 64 + BL * a : s * 64 + BL * b].rearrange(
                                "p (j c) -> p j c", j=b - a
                            ),
                        )

                for s in range(nsteps_blk):
                    i = i0 + s
                    tb = unpool.tile([128, 512], f32, tag="tb")
                    ur = urpool.tile([128, 512], b16, tag="ur")
                    # single phase, N=512 streams (a two-phase N=256 column
                    # split that hides the transpose chain measured WORSE:
                    # +415ns round-boundary cost vs ~260ns of hiding gained)
                    P = prpool.tile([128, 512], f32, tag="P")
                    for k in range(NK - 1):
                        for g in range(NG):
                            nc.tensor.matmul(
                                P[32 * g : 32 * g + BL, :],
                                lhsT(i, k),
                                a_sb[:, k * V + g * 512 : k * V + (g + 1) * 512],
                                start=(k == 0),
                                stop=False,
                                tile_position=(0, 32 * g),
                            )
                    # final round split into column halves: the first half's
                    # completion releases the transposes for k-tiles 0..7 one
                    # half-round early, overlapping the chain head with the
                    # PE's last 256 columns of streaming
                    kl = NK - 1
                    for h in range(2):
                        for g in range(NG):
                            nc.tensor.matmul(
                                P[32 * g : 32 * g + BL, h * 256 : (h + 1) * 256],
                                lhsT(i, kl),
                                a_sb[
                                    :,
                                    kl * V + g * 512 + h * 256 :
                                    kl * V + g * 512 + (h + 1) * 256,
                                ],
                                start=False,
                                stop=True,
                                tile_position=(0, 32 * g),
                                skip_group_check=True,
                            )
                    if nodve:
                        if probe:
                            # dangling transposes: read PSUM, no consumers —
                            # isolates DVE-PSUM contention from dependencies
                            for a, b in ((0, 4), (4, 8), (8, 12), (12, 16)):
                                nc.vector.transpose(
                                    tb[:, 32 * a : 32 * b], P[:, 32 * a : 32 * b]
                                )
                        state["prev"] = u_init
                        nc.scalar.copy(
                            stage[:, s * 64 : (s + 1) * 64], u_init[:, 0:64]
                        )
                        continue
                    # uniform 4-piece split measured best: finer head pieces
                    # cost more in DVE op overhead than they save in latency
                    pieces(P, tb, ur, s,
                           ((0, 4), (4, 8), (8, 12), (12, 16)), 0)
                    state["prev"] = ur
                    nc.scalar.copy(
                        stage[:, s * 64 : (s + 1) * 64].rearrange(
                            "p (j c) -> p j c", j=NK
                        ),
                        ur[:, :].rearrange("p (j c) -> p j c", j=NK)[:, :, 0:BL],
                    )
                o0 = 0 if timing_mode else i0 * 128
                # scalar (ACT) DGE queue: gpsimd-queue DMA triggers are Pool
                # ISA instructions and break walrus codegen inside For_i
                nc.scalar.dma_start(
                    uh_out[o0 : o0 + nsteps_blk * 128, :].rearrange(
                        "(s p) c -> p s c", s=nsteps_blk
                    ),
                    stage[:, 0 : nsteps_blk * 64].rearrange(
                        "p (s c) -> p s c", s=nsteps_blk
                    ),
                )

            def run_all():
                load_x(0)
                for blk in range(nb):
                    body(blk)

            if outer_reps == 1:
                run_all()
            else:
                with tc.For_i(0, outer_reps):
                    run_all()
    if split_waits:
        _split_multi_waits(nc)
    return nc


def _prep_host(inputs, n_steps):
    """Host-side preprocessing shared by all cores."""
    x = np.asarray(inputs["extracted_log_probs"], np.float32)   # [V,B,T]
    in_idxs = np.asarray(inputs["in_idxs"]).astype(np.int64)
    out_idxs = np.asarray(inputs["out_idxs"]).astype(np.int64)
    start_idxs = np.asarray(inputs["start_idxs"]).astype(np.int64)
    end_idxs = np.asarray(inputs["end_idxs"]).astype(np.int64)

    xt = np.ascontiguousarray(np.transpose(x, (2, 1, 0)))       # [T,B,V]

    A_cnt = np.zeros((V, V), np.float32)
    np.add.at(A_cnt, (out_idxs, in_idxs), 1.0)

    end_w = np.zeros((V,), np.float32)
    np.add.at(end_w, end_idxs, 1.0)

    start_mask = np.zeros((V,), bool)
    start_mask[start_idxs] = True

    # A tiles, row-permuted to the v-major state layout:
    # a_sb[32g + r, k*V + w] = A[512g + 32k + r, w]
    a_sb = np.ascontiguousarray(
        A_cnt.reshape(NG, NK, 32, V).transpose(0, 2, 1, 3).reshape(128, NK * V)
    ).astype(bf16)

    # U_0 = exp(log_curr0)
    X0 = np.exp(xt[0])                                           # [B,V]
    U0 = np.where(start_mask[None, :], X0, np.float32(EPS)).astype(np.float32)
    U0_16 = U0.astype(bf16)

    # scales sigma_t[b] folded into X' (t = 1..n_steps)
    Xall = np.exp(xt[1 : n_steps + 1])                           # [n,B,V]
    m = Xall.mean(axis=2)                                        # [n,B]
    sigma = (1.0 / (8.0 * m)).astype(np.float32)
    cumlog = np.cumsum(np.log(sigma.astype(np.float64)), axis=0) # [n,B]
    Xs16 = (Xall * sigma[:, :, None]).astype(bf16)               # [n,B,V]

    return dict(a_sb=a_sb, U0_16=U0_16, Xs16=Xs16, cumlog=cumlog, end_w=end_w)


def _core_inputs(prep, core, n_steps):
    bsl = slice(core * BL, (core + 1) * BL)
    # u0t[32g + r, 32j + b] = U0[b, 512g + 32j + r], junk columns zero
    u0c = prep["U0_16"][bsl]                                     # [BL, V]
    u0t = np.zeros((NG, 32, NK, 32), bf16)                       # [g, r, j, c]
    u0t[:, :, :, :BL] = u0c.reshape(BL, NG, NK, 32).transpose(1, 3, 2, 0)
    u0t = np.ascontiguousarray(u0t.reshape(128, 512))
    # xs[(t, 32g + r), 4j + b] = Xs16[t, b, 512g + 32j + r]
    xc = prep["Xs16"][:, bsl, :]                                 # [n, BL, V]
    xs = np.ascontiguousarray(
        xc.reshape(n_steps, BL, NG, NK, 32)
        .transpose(0, 2, 4, 3, 1)                                # [t, g, r, j, b]
        .reshape(n_steps * 128, NK * BL)
    )
    return {"a_rhs": prep["a_sb"], "u0t": u0t, "xs": xs}


def _postprocess(prep, results, target_lengths, n_steps):
    """results: list of per-core out_maps with 'u_hist'."""
    end_w_gjr = prep["end_w"].reshape(NG, NK, 32)                # [g, j, r]
    E_dev = np.zeros((n_steps + 1, B), np.float64)
    # t = 0 from host U0 (bf16-rounded, same as device state precision)
    E_dev[0] = prep["U0_16"].astype(np.float32) @ prep["end_w"]
    for c in range(NCORES):
        uh = np.asarray(results[c]["u_hist"]).reshape(n_steps, NG, 32, NK, BL)
        # E[t, b] = sum_{g,r,j} uh[t, g, r, j, b] * end_w[g, j, r]
        Ec = np.einsum("tgrjb,gjr->tb", uh.astype(np.float32), end_w_gjr)
        E_dev[1:, c * BL : (c + 1) * BL] = Ec
    lengths = np.asarray(target_lengths).astype(np.int64)
    res = np.zeros((B,), np.float64)
    for b in range(B):
        L = int(lengths[b])
        corr = prep["cumlog"][L - 2, b] if L >= 2 else 0.0
        res[b] = np.log(E_dev[L - 1, b]) - corr
    return (-res).astype(np.float32)


def run_on_device(nc, core_maps, **kwargs):
    from concourse.bass_utils import run_bass_kernel_spmd

    return run_bass_kernel_spmd(nc, core_maps, core_ids=list(range(NCORES)), **kwargs)


def kernel(**inputs) -> np.ndarray:
    lengths = np.asarray(inputs["target_lengths"]).astype(np.int64)
    n_steps = max(1, int(lengths.max()) - 1)
    prep = _prep_host(inputs, n_steps)
    core_maps = [_core_inputs(prep, c, n_steps) for c in range(NCORES)]
    last_err = None
    for attempt in range(3):
        try:
            if n_steps not in _PROGRAM_CACHE:
                _PROGRAM_CACHE[n_steps] = build_program(n_steps)
            nc = _PROGRAM_CACHE[n_steps]
            out = run_on_device(nc, core_maps)
            break
        except Exception as e:                      # flaky axon compile path
            last_err = e
            _PROGRAM_CACHE.pop(n_steps, None)
    else:
        raise last_err
    return _postprocess(prep, out.results, inputs["target_lengths"], n_steps)
